# revision 55
# baseline (speedup 1.0000x reference)
"""Brownian-bridge criterion loss on 8 Trainium2 NeuronCores — fused
single-launch kernel.

Strategy (data-parallel over the n = bs*q sequence axis, hint-compliant):
  Host (indexing only): sort sequences by bridge pivot; core k owns
  sorted cur sequences [200k, 200k+200) and 200 other sequences. Inputs
  are staged transposed (bf16) as one [256, 9400] tensor per core:
    [head/tail frames of ALL 1600 sequences, interleaved in (g0, g2)
     pairs of 512 | own pivot frames | own g0/g2 copies |
     g=1..14: (own cur-f_g, own oth-f_g) pool groups of 400]
  Replicating the head/tail columns lets every core build the full
  1600-column A matrix locally, so the kernel needs no collective
  before the cross matmuls (the collective entry latency here is large
  and highly variable).

  Device (one launch per core):
   1. Transposed projection embT = W^T x + b with W stationary,
      software-pipelined in 512-column chunks; per-column L2 norm via
      Square + ones-matmul partition reduction + Abs_reciprocal_sqrt +
      K=1 broadcast matmul; bias+normalize fused into one
      scalar_tensor_tensor per half (bf16 embeddings).
   2. A-matrix slices aT = (1-alpha) g0 + alpha g2 are built as each
      (g0, g2) pair chunk retires; alpha broadcast via gpsimd
      partition_broadcast. Per-seq dots (q, aa, score) via ones-matmul;
      self-dist s, numer, c0, c1, softplus head-tail term.
   3. Cross matmuls per pivot group (exact value-dependent bounds) are
      interleaved into the projection stream as each pool group's
      normalize retires: cross = A_g^T @ pool_g -> Max8 = this core's
      top-8 candidates per global row (stored as bf16).
   4. One packed AllGather of [scalars (200x5 f32); top8 (1600x8 bf16)]
      per rank; a tiny dummy AllGather at kernel start absorbs the
      first-collective entry latency off the critical path.
   5. Replicated final phase, batched over all 13 row tiles: dist =
      c1*cross + c0 (monotone per row), top-8 of 64, deno = numer +
      sum(exp(top6)) - exp(max(s, v6)) (arithmetic self-exclusion),
      means via ones-matmul; core 0's [1, 2] output is the answer.

The instruction stream depends on the bridge contents (group bounds),
so compiled NEFFs are cached keyed by a hash of the bridge tensor and
rebuilt if it changes.
"""

import hashlib
import sys

sys.path.insert(0, "/opt/trn_rl_repo")

import ml_dtypes
import numpy as np

import concourse.bacc as bacc
import concourse.bass as bass
import concourse.mybir as mybir
import concourse.tile as tile
from concourse.bass_utils import run_bass_kernel_spmd

F32 = mybir.dt.float32
F32R = mybir.dt.float32r
BF16 = mybir.dt.bfloat16
I32 = mybir.dt.int32
AF = mybir.ActivationFunctionType
OP = mybir.AluOpType
AX = mybir.AxisListType

BS, T, Q, HID, PROJ = 16, 16, 100, 256, 256
NSEQ = BS * Q              # 1600
NCORES = 8
SPC = NSEQ // NCORES       # 200 cur sequences per core
NG = T - 2                 # 14 pivot groups
HT2 = NSEQ                 # g2_all block start (g0_all at 0)
PV = 2 * NSEQ              # own pivot-frame block (200)
OG0 = PV + SPC             # own g0 copy (200)
OG2 = OG0 + SPC            # own g2 copy (200)
POOL0 = OG2 + SPC          # 3800: pool region start
C = POOL0 + 2 * SPC * NG   # 9400 columns per core
DELTA = 0.3
CHUNK = 512
ROWT = (NSEQ + 127) // 128  # 13 final-phase row tiles

RSPLIT = NSEQ             # AG2 split row (NSEQ = single collective)
CCA_LEN = SPC * 5 + 4 * RSPLIT       # scal (f32) + t8 rows (bf16 pairs)
CCB_LEN = 4 * (NSEQ - RSPLIT)


def _chunks():
    out = []
    s = 0
    while s < C:
        out.append((s, min(CHUNK, C - s)))
        s += CHUNK
    return out


def _build_fused(gbounds):
    """gbounds: list of (gs, ge) global sorted-row bounds per group g=1..14."""
    nc = bacc.Bacc("TRN2", target_bir_lowering=False, debug=False,
                   num_devices=NCORES)
    xt_in = nc.declare_dram_parameter("xt_in", [HID, C], BF16, isOutput=False)
    w_in = nc.declare_dram_parameter("w_in", [HID, PROJ], BF16, isOutput=False)
    b_in = nc.declare_dram_parameter("b_in", [HID, 1], F32, isOutput=False)
    brt_in = nc.declare_dram_parameter("brt_in", [3, SPC], I32, isOutput=False)
    brta_in = nc.declare_dram_parameter("brta_in", [3, NSEQ], I32,
                                        isOutput=False)
    ones_in = nc.declare_dram_parameter("ones_in", [2, 128], F32R,
                                        isOutput=False)
    out2 = nc.declare_dram_parameter("out2", [1, 2], F32, isOutput=True)

    # value-dependent M-tiles: (group g, global row start, row end)
    mtiles = []
    for g in range(1, NG + 1):
        gs, ge = gbounds[g - 1]
        ms = gs
        while ms < ge:
            me = min(ms + 128, ge)
            mtiles.append((g, ms, me))
            ms = me

    with tile.TileContext(nc) as tc:
        with (
            tc.tile_pool(name="singles", bufs=1) as singles,
            tc.tile_pool(name="work", bufs=4) as work,
            tc.tile_pool(name="fin", bufs=3) as finp,
            tc.tile_pool(name="dram", bufs=1, space="DRAM") as dram,
        ):
            engs = (nc.sync, nc.scalar, nc.gpsimd)

            # ---- activation table preheat (overlaps input DMA) ----
            dummy = singles.tile([1, 8], F32, tag="dummy")
            nc.vector.memset(dummy, 1.0)
            for fn in (AF.Exp, AF.Abs_reciprocal_sqrt):
                nc.scalar.activation(out=dummy, in_=dummy, func=fn)

            # ---- constants / small inputs ----
            w_sb = []
            for kt in range(2):
                t_w = singles.tile([128, PROJ], BF16, tag=f"w{kt}")
                engs[kt].dma_start(out=t_w, in_=w_in[kt * 128:(kt + 1) * 128, :])
                w_sb.append(t_w)
            ones128 = singles.tile([128, 1], F32R, tag="ones128")
            nc.gpsimd.dma_start(
                out=ones128,
                in_=ones_in[0:1, :].rearrange("x (b y) -> (x b) y", y=1))
            ones1 = singles.tile([1, 128], F32R, tag="ones1")
            nc.gpsimd.dma_start(out=ones1, in_=ones_in[0:1, :])
            delta1 = singles.tile([1, 1], F32, tag="delta1")
            nc.vector.memset(delta1, DELTA)
            b_sb = []
            for h in range(2):
                t_b = singles.tile([128, 1], F32, tag=f"b{h}")
                nc.gpsimd.dma_start(out=t_b, in_=b_in[h * 128:(h + 1) * 128, :])
                b_sb.append(t_b)

            # ---- xt input: graded slabs, both halves' heads first ----
            xt_sb = [singles.tile([128, C], BF16, tag=f"xt{h}", name=f"xt{h}")
                     for h in range(2)]
            SLABS = (512, 512, 1024, 1536, 1536, 1536, 1372, 1372)
            a = 0
            for si, w_s in enumerate(SLABS):
                bnd = min(a + w_s, C)
                for h in range(2):
                    eng = nc.gpsimd if si == 3 else engs[h]
                    eng.dma_start(
                        out=xt_sb[h][:, a:bnd],
                        in_=xt_in[h * 128:(h + 1) * 128, a:bnd])
                a = bnd

            emb = [singles.tile([128, C], BF16, tag=f"emb{h}", name=f"emb{h}")
                   for h in range(2)]
            aT_all = [singles.tile([128, NSEQ], BF16, tag=f"aTall{h}",
                                   name=f"aTall{h}") for h in range(2)]

            # dummy early AllGather: absorbs the (large, variable)
            # first-collective entry latency off the critical path; its
            # completion is never awaited by compute
            ccW_in = dram.tile([1, 32], F32, tag="ccWin")
            ccW_out = dram.tile([NCORES, 32], F32, tag="ccWout",
                                addr_space="Shared")
            warm = singles.tile([1, 32], F32, tag="warm")
            nc.vector.memset(warm, 0.0)
            nc.gpsimd.dma_start(out=ccW_in[:, :], in_=warm)
            nc.gpsimd.collective_compute(
                "AllGather", OP.bypass,
                replica_groups=[list(range(NCORES))],
                ins=[ccW_in[:, :].opt()],
                outs=[ccW_out[:, :].opt()])
            cc_inA = dram.tile([CCA_LEN, 1], F32, tag="ccinA")
            cc_outA = dram.tile([NCORES, CCA_LEN, 1], F32, tag="ccoutA",
                                addr_space="Shared")
            if RSPLIT < NSEQ:
                cc_inB = dram.tile([CCB_LEN, 1], F32, tag="ccinB")
                cc_outB = dram.tile([NCORES, CCB_LEN, 1], F32, tag="ccoutB",
                                    addr_space="Shared")

            # ---- per-seq alpha chain (needs only brt, runs before proj) ----
            bfi = singles.tile([1, 3 * SPC], I32, tag="bfi")
            for i in range(3):
                nc.gpsimd.dma_start(out=bfi[:, i * SPC:(i + 1) * SPC],
                                    in_=brt_in[i:i + 1, :])
            bf = singles.tile([1, 3 * SPC], F32, tag="bf")
            nc.vector.tensor_copy(out=bf, in_=bfi)
            bh, bp, bt = (bf[:, i * SPC:(i + 1) * SPC] for i in range(3))
            sc = singles.tile([1, 2600], F32, tag="sc")

            def R(i):
                return sc[:, i * SPC:(i + 1) * SPC]

            (alpha, rsg, s_sd, c0r, nmr, spr, tmp, sigr, qr, aar,
             scr) = (R(i) for i in range(11))

            def tt(o, i0, i1, op):
                nc.vector.tensor_tensor(out=o, in0=i0, in1=i1, op=op)

            tt(alpha, bp, bh, OP.subtract)
            tt(tmp, bt, bh, OP.subtract)
            nc.vector.reciprocal(out=tmp, in_=tmp)
            tt(alpha, alpha, tmp, OP.mult)
            tt(sigr, bt, bp, OP.subtract)
            tt(sigr, alpha, sigr, OP.mult)          # sigma
            tt(tmp, sigr, sigr, OP.mult)            # sigma^2
            nc.vector.reciprocal(out=rsg, in_=tmp)  # c1 = 1/sigma^2

            # global alpha over all 1600 sorted rows (identical on all
            # cores); bridge head/tail are 0/T-1 by construction, so
            # alpha = pivot/(T-1)
            bfa_i = work.tile([1, NSEQ], I32, tag="bfai", bufs=1)
            nc.gpsimd.dma_start(out=bfa_i, in_=brta_in[1:2, :])
            al_a = singles.tile([1, NSEQ], F32, tag="ala")
            nc.vector.tensor_copy(out=al_a, in_=bfa_i)
            nc.vector.tensor_scalar(out=al_a, in0=al_a,
                                    scalar1=1.0 / (T - 1), scalar2=None,
                                    op0=OP.mult)

            with tc.tile_pool(name="psA", bufs=1, space="PSUM") as psA:
                ab = singles.tile([128, SPC], F32, tag="ab", name="ab")
                nc.gpsimd.partition_broadcast(ab[:, :], alpha[:, :])
                ab_all = singles.tile([128, NSEQ], F32, tag="aball",
                                      name="ab_all")
                nc.gpsimd.partition_broadcast(ab_all[:, :], al_a[:, :])
                om_a = work.tile([1, NSEQ], F32, tag="oma", bufs=1)
                nc.vector.tensor_scalar(out=om_a, in0=al_a, scalar1=-1.0,
                                        scalar2=1.0, op0=OP.mult, op1=OP.add)
                omb_all = singles.tile([128, NSEQ], F32, tag="omball",
                                       name="omb_all")
                nc.gpsimd.partition_broadcast(omb_all[:, :], om_a[:, :])

                # software-pipelined projection: stage A (proj matmuls +
                # squares) for chunk c+1 is emitted before stage B
                # (colsum/rsqrt/bcast/normalize) of chunk c, so the PE
                # queue never blocks on the scalar engine.
                def stage_a(s, w):
                    ps_p = []
                    sq = []
                    for h in range(2):
                        pp = psA.tile([128, CHUNK], F32, tag=f"pp{h}", bufs=2)
                        for kt in range(2):
                            nc.tensor.matmul(
                                out=pp[:, :w],
                                lhsT=w_sb[kt][:, h * 128:(h + 1) * 128],
                                rhs=xt_sb[kt][:, s:s + w],
                                start=(kt == 0), stop=(kt == 1))
                        sq_h = work.tile([128, CHUNK], F32R, tag=f"sq{h}", bufs=3)
                        nc.scalar.activation(out=sq_h[:, :w], in_=pp[:, :w],
                                             func=AF.Square, bias=b_sb[h])
                        ps_p.append(pp)
                        sq.append(sq_h)
                    return (s, w, ps_p, sq)

                def stage_b(st):
                    s, w, ps_p, sq = st
                    ss = psA.tile([1, CHUNK], F32, tag="ss", bufs=1)
                    for h in range(2):
                        nc.tensor.matmul(out=ss[:, :w], lhsT=ones128,
                                         rhs=sq[h][:, :w],
                                         start=(h == 0), stop=(h == 1))
                    rn = work.tile([1, CHUNK], F32R, tag="rn", bufs=3)
                    nc.scalar.activation(out=rn[:, :w], in_=ss[:, :w],
                                         func=AF.Abs_reciprocal_sqrt)
                    rb = psA.tile([128, CHUNK], F32, tag="rb", bufs=1)
                    nc.tensor.matmul(out=rb[:, :w], lhsT=ones1, rhs=rn[:, :w],
                                     start=True, stop=True)
                    rb_sb = work.tile([128, CHUNK], F32, tag="rbsb", bufs=3)
                    nc.vector.tensor_copy(out=rb_sb[:, :w], in_=rb[:, :w])
                    with nc.allow_low_precision(reason="bf16 embeddings"):
                        for h in range(2):
                            nc.vector.scalar_tensor_tensor(
                                out=emb[h][:, s:s + w], in0=ps_p[h][:, :w],
                                scalar=b_sb[h], in1=rb_sb[:, :w],
                                op0=OP.add, op1=OP.mult)

                chunks = _chunks()
                # group g's pool columns end at POOL0 + 400g; map each
                # chunk to the groups whose pool it completes
                gdone = {}
                for g in range(1, NG + 1):
                    cidx = (POOL0 + 2 * SPC * g - 1) // CHUNK
                    gdone.setdefault(cidx, []).append(g)
                xei = 0

                def emit_cross(g):
                    nonlocal xei
                    for (gg, ms, me) in mtiles:
                        if gg != g:
                            continue
                        cw = me - ms
                        px = psA.tile([128, 2 * SPC], F32, tag="px", bufs=2,
                                      padded_shape=[128, CHUNK])
                        pool = POOL0 + 2 * SPC * (g - 1)
                        for h in range(2):
                            nc.tensor.matmul(
                                out=px[:cw, :],
                                lhsT=aT_all[h][:, ms:me],
                                rhs=emb[h][:, pool:pool + 2 * SPC],
                                start=(h == 0), stop=(h == 1))
                        t8t = work.tile([128, 8], BF16, tag="t8t")
                        nc.vector.max(out=t8t[:cw, :], in_=px[:cw, :])
                        engs[xei % 2].dma_start(
                            out=cc_inA[SPC * 5 + 4 * ms:SPC * 5 + 4 * me,
                                       0:1].rearrange(
                                           "(s e) x -> s (x e)",
                                           e=4).bitcast(BF16),
                            in_=t8t[:cw, :])
                        xei += 1

                pend = None
                aT = []
                for ci, (s, w) in enumerate(chunks):
                    cur = stage_a(s, w)
                    if pend is not None:
                        stage_b(pend)
                        bci = ci - 1
                        if bci >= 8:
                            for g in gdone.get(bci, []):
                                emit_cross(g)
                    pend = cur
                    if ci in (2, 4, 6, 7):
                        # the (g0, g2) pair covering aT columns
                        # [512j, 512j+PW) is normalized: build that slice of
                        # the global A matrix (small DVE ops, pipelined)
                        j = ci // 2 - 1 if ci < 7 else 3
                        a0 = 512 * j
                        pw = min(512, NSEQ - a0)
                        for h in range(2):
                            g0j = emb[h][:, 1024 * j:1024 * j + pw]
                            g2j = emb[h][:, 1024 * j + pw:1024 * j + 2 * pw]
                            da = work.tile([128, CHUNK], F32, tag=f"da{h}",
                                           bufs=2)
                            db = work.tile([128, CHUNK], F32, tag=f"db{h}",
                                           bufs=2)
                            tt(da[:, :pw], g0j,
                               omb_all[:, a0:a0 + pw], OP.mult)
                            tt(db[:, :pw], g2j,
                               ab_all[:, a0:a0 + pw], OP.mult)
                            with nc.allow_low_precision(reason="bf16 A"):
                                tt(aT_all[h][:, a0:a0 + pw], da[:, :pw],
                                   db[:, :pw], OP.add)
                    if ci == 8:
                        # own copies (cols 3200:3800) are normalized:
                        # per-seq dots and scalars
                        for h in range(2):
                            g0o = emb[h][:, OG0:OG0 + SPC]
                            g2o = emb[h][:, OG2:OG2 + SPC]
                            d = work.tile([128, SPC], F32, tag=f"ad{h}", bufs=1)
                            tt(d, g2o, g0o, OP.subtract)
                            a_h = singles.tile([128, SPC], F32R, tag=f"aT{h}",
                                               name=f"aTh{h}")
                            nc.vector.tensor_tensor(out=a_h, in0=d, in1=ab,
                                                    op=OP.mult)
                            tt(a_h, a_h, g0o, OP.add)
                            aT.append(a_h)
                        # dots: q = a.g1, aa = a.a, score = g0.g2
                        for di, (f0, f1) in enumerate((
                            (lambda h: aT[h],
                             lambda h: emb[h][:, PV:PV + SPC]),
                            (lambda h: aT[h], lambda h: aT[h]),
                            (lambda h: emb[h][:, OG0:OG0 + SPC],
                             lambda h: emb[h][:, OG2:OG2 + SPC]),
                        )):
                            dp_t = psA.tile([1, CHUNK], F32, tag="rb",
                                            bufs=1, name="dp_t")
                            dp = dp_t[:, :SPC]
                            for h in range(2):
                                pr = work.tile([128, SPC], F32R, tag=f"pr{h}", bufs=2)
                                tt(pr, f0(h), f1(h), OP.mult)
                                nc.tensor.matmul(out=dp, lhsT=ones128,
                                                 rhs=pr,
                                                 start=(h == 0), stop=(h == 1))
                            nc.vector.tensor_copy(out=R(8 + di), in_=dp)

                        # s = (2q - 1 - aa)/(2 sigma^2)
                        nc.vector.tensor_scalar(out=tmp, in0=qr, scalar1=2.0,
                                                scalar2=-1.0, op0=OP.mult,
                                                op1=OP.add)
                        tt(tmp, tmp, aar, OP.subtract)
                        tt(tmp, tmp, rsg, OP.mult)
                        nc.vector.tensor_scalar(out=s_sd, in0=tmp, scalar1=0.5,
                                                scalar2=None, op0=OP.mult)
                        # c0 = -(1 + aa)/(2 sigma^2)
                        nc.vector.tensor_scalar(out=tmp, in0=aar, scalar1=1.0,
                                                scalar2=None, op0=OP.add)
                        tt(tmp, tmp, rsg, OP.mult)
                        nc.vector.tensor_scalar(out=c0r, in0=tmp, scalar1=-0.5,
                                                scalar2=None, op0=OP.mult)
                if pend is not None:
                    stage_b(pend)
                    for g in gdone.get(len(chunks) - 1, []):
                        emit_cross(g)

                # exp/ln work deferred here so the scalar engine never swaps
                # activation tables mid-projection
                nc.scalar.activation(out=nmr, in_=s_sd, func=AF.Exp)
                nc.scalar.activation(out=spr, in_=scr, func=AF.Exp,
                                     scale=-1.0, bias=delta1)
                nc.scalar.activation(out=spr, in_=spr, func=AF.Ln,
                                     bias=1.0)
                scv = cc_inA[0:SPC * 5, 0:1].rearrange(
                    "(s e) x -> e (s x)", e=5)
                for qi, row in enumerate((c0r, rsg, s_sd, nmr, spr)):
                    nc.scalar.dma_start(out=scv[qi:qi + 1, :], in_=row)

            with tc.tile_pool(name="psB", bufs=1, space="PSUM") as psB:
                # ---- AllGather 2 (cross already ran inline above) ----
                nc.gpsimd.collective_compute(
                    "AllGather", OP.bypass,
                    replica_groups=[list(range(NCORES))],
                    ins=[cc_inA[:, :].opt()],
                    outs=[cc_outA[:, :, :].opt()])

                # ---- final phase (replicated, batched over row tiles) ----
                cand_all = singles.tile([128, ROWT, 64], BF16, tag="candall")
                sct_all = singles.tile([128, ROWT, 5], F32, tag="sctall")
                nc.vector.memset(cand_all, 0.0)
                nc.vector.memset(sct_all, 0.0)

                def cc_view(t):
                    r0 = 128 * t
                    psz = min(128, NSEQ - r0)
                    return cc_outA[:, SPC * 5 + 4 * r0:
                                   SPC * 5 + 4 * (r0 + psz), 0:1].rearrange(
                                       "k (s e) x -> s k (e x)",
                                       e=4).bitcast(BF16)
                ei = 0
                for t in range(ROWT):
                    r0 = 128 * t
                    psz = min(128, NSEQ - r0)
                    engs[ei % 3].dma_start(
                        out=cand_all[:psz, t, :].rearrange(
                            "s (k e) -> s k e", k=8),
                        in_=cc_view(t))
                    ei += 1
                    k0 = r0 // SPC
                    k1 = (r0 + psz - 1) // SPC
                    for k in range(k0, k1 + 1):
                        a = max(r0, SPC * k)
                        bnd = min(r0 + psz, SPC * (k + 1))
                        engs[ei % 3].dma_start(
                            out=sct_all[a - r0:bnd - r0, t, :],
                            in_=cc_outA[k, (a - SPC * k) * 5:
                                        (bnd - SPC * k) * 5,
                                        0:1].rearrange("(s e) x -> s (x e)", e=5))
                        ei += 1
                c0b = sct_all[:, :, 0:1].to_broadcast([128, ROWT, 64])
                c1b = sct_all[:, :, 1:2].to_broadcast([128, ROWT, 64])
                d_all = singles.tile([128, ROWT, 64], F32, tag="dall")
                nc.vector.tensor_tensor(out=d_all, in0=cand_all, in1=c1b,
                                        op=OP.mult)
                nc.vector.tensor_tensor(out=d_all, in0=d_all, in1=c0b,
                                        op=OP.add)
                t8a = singles.tile([128, ROWT, 8], F32, tag="t8a")
                for t in range(ROWT):
                    nc.vector.max(out=t8a[:, t, :], in_=d_all[:, t, :])
                e6 = singles.tile([128, ROWT, 6], F32, tag="e6")
                nc.scalar.activation(out=e6, in_=t8a[:, :, 0:6], func=AF.Exp)
                se = singles.tile([128, ROWT], F32, tag="se")
                nc.vector.reduce_sum(out=se[:, :].unsqueeze(-1), in_=e6,
                                     axis=AX.X)
                mx = singles.tile([128, ROWT], F32, tag="mx")
                nc.vector.tensor_tensor(out=mx[:, :].unsqueeze(-1),
                                        in0=t8a[:, :, 5:6],
                                        in1=sct_all[:, :, 2:3], op=OP.max)
                em = singles.tile([128, ROWT], F32, tag="em")
                nc.scalar.activation(out=em, in_=mx, func=AF.Exp)
                nmv = sct_all[:, :, 3]
                nc.vector.tensor_tensor(out=se, in0=se, in1=em, op=OP.subtract)
                nc.vector.tensor_tensor(out=se[:, :].unsqueeze(-1),
                                        in0=se[:, :].unsqueeze(-1),
                                        in1=sct_all[:, :, 3:4], op=OP.add)
                nc.vector.reciprocal(out=se, in_=se)
                lsum = singles.tile([128, ROWT], F32, tag="lsum")
                nc.vector.tensor_tensor(out=lsum[:, :].unsqueeze(-1),
                                        in0=sct_all[:, :, 3:4],
                                        in1=se[:, :].unsqueeze(-1), op=OP.mult)

                red = singles.tile([128, 2], F32R, tag="red")
                with nc.allow_low_precision(reason="f32r is f32 bits"):
                    nc.vector.reduce_sum(out=red[:, 0:1], in_=lsum, axis=AX.X)
                    nc.vector.reduce_sum(out=red[:, 1:2],
                                     in_=sct_all[:, :, 4:5].rearrange(
                                         "p t x -> p (t x)"), axis=AX.X)
                fin_ps = psB.tile([1, 2], F32, tag="finps", bufs=1,
                                  padded_shape=[1, CHUNK])
                nc.tensor.matmul(out=fin_ps, lhsT=ones128,
                                 rhs=red,
                                 start=True, stop=True)
                fin = singles.tile([1, 2], F32, tag="fin")
                nc.vector.tensor_scalar(out=fin, in0=fin_ps,
                                        scalar1=1.0 / NSEQ,
                                        scalar2=None, op0=OP.mult)
                nc.sync.dma_start(out=out2[:, :], in_=fin)
    nc.compile()
    return nc


_NC_CACHE = {}
LAST_RUNS = []


def _hw_runner(nc, in_maps):
    import os
    res = run_bass_kernel_spmd(
        nc, in_maps, list(range(NCORES)),
        trace=bool(os.environ.get("KERNEL_TRACE")))
    LAST_RUNS.append(res)
    return res.results


def kernel(frame_embeds, other_frame_embeds, W, b, bridge, _runner=None):
    frame_embeds = np.asarray(frame_embeds, dtype=np.float32)
    other_frame_embeds = np.asarray(other_frame_embeds, dtype=np.float32)
    W = np.ascontiguousarray(np.asarray(W, dtype=np.float32))
    b = np.asarray(b, dtype=np.float32)
    bridge = np.asarray(bridge, dtype=np.int32)

    runner = _runner if _runner is not None else _hw_runner

    # ---- host-side sharding / layout (pure indexing) ----
    fe_seq = frame_embeds.transpose(0, 2, 1, 3).reshape(NSEQ, T, HID)
    ofe_seq = other_frame_embeds.transpose(0, 2, 1, 3).reshape(NSEQ, T, HID)
    perm = np.argsort(bridge[:, 1], kind="stable")
    bridge_s = bridge[perm]

    piv = bridge_s[:, 1].astype(np.int64)
    counts = np.bincount(piv, minlength=T)[1:T - 1]
    gb = np.zeros(NG + 1, dtype=np.int64)
    gb[1:] = np.cumsum(counts)
    gbounds = [(int(gb[g - 1]), int(gb[g])) for g in range(1, NG + 1)]

    key = ("fused", hashlib.sha1(bridge.tobytes()).hexdigest())
    if key not in _NC_CACHE:
        _NC_CACHE[key] = _build_fused(gbounds)
    nc = _NC_CACHE[key]

    b_col = np.ascontiguousarray(b.reshape(HID, 1))
    ones_host = np.ones((2, 128), np.float32)
    W_bf = W.astype(ml_dtypes.bfloat16)
    fe_sorted = fe_seq[perm]                         # (1600, 16, 256)
    g0_all = fe_sorted[:, 0, :].T                    # (256, 1600)
    g2_all = fe_sorted[:, T - 1, :].T
    brtA = np.ascontiguousarray(bridge_s.T)          # (3, 1600)
    in_maps = []
    for k in range(NCORES):
        sl = slice(k * SPC, (k + 1) * SPC)
        fe_k = fe_sorted[sl]                         # (200, 16, 256)
        cur_t = fe_k.transpose(2, 1, 0)              # (256, 16, 200)
        oth_t = ofe_seq[sl].transpose(2, 1, 0)
        X = np.empty((HID, C), np.float32)
        for j in range(4):
            a0 = 512 * j
            pw = min(512, NSEQ - a0)
            X[:, 1024 * j:1024 * j + pw] = g0_all[:, a0:a0 + pw]
            X[:, 1024 * j + pw:1024 * j + 2 * pw] = g2_all[:, a0:a0 + pw]
        X[:, PV:PV + SPC] = fe_k[np.arange(SPC), bridge_s[sl, 1]].T
        X[:, OG0:OG0 + SPC] = cur_t[:, 0]
        X[:, OG2:OG2 + SPC] = cur_t[:, T - 1]
        for g in range(1, NG + 1):
            base = POOL0 + 2 * SPC * (g - 1)
            X[:, base:base + SPC] = cur_t[:, g]
            X[:, base + SPC:base + 2 * SPC] = oth_t[:, g]
        brT = np.ascontiguousarray(bridge_s[sl].T)
        in_maps.append({"xt_in": X.astype(ml_dtypes.bfloat16), "w_in": W_bf,
                        "b_in": b_col, "brt_in": brT, "brta_in": brtA,
                        "ones_in": ones_host})

    res = runner(nc, in_maps)
    out = res[0]["out2"]
    return (np.asarray(np.float32(out[0, 0])), np.asarray(np.float32(out[0, 1])))


# revision 56
# speedup vs baseline: 1.0865x; 1.0865x over previous
"""Brownian-bridge criterion loss on 8 Trainium2 NeuronCores — fused
single-launch kernel.

Strategy (data-parallel over the n = bs*q sequence axis, hint-compliant):
  Host (indexing only): sort sequences by bridge pivot; core k owns
  sorted cur sequences [200k, 200k+200) and 200 other sequences. Inputs
  are staged transposed (bf16) as one [256, 9400] tensor per core:
    [head/tail frames of ALL 1600 sequences, interleaved in (g0, g2)
     pairs of 512 | own pivot frames | own g0/g2 copies |
     g=1..14: (own cur-f_g, own oth-f_g) pool groups of 400]
  Replicating the head/tail columns lets every core build the full
  1600-column A matrix locally, so the kernel needs no collective
  before the cross matmuls (the collective entry latency here is large
  and highly variable).

  Device (one launch per core):
   1. Transposed projection embT = W^T x + b with W stationary,
      software-pipelined in 512-column chunks; per-column L2 norm via
      Square + ones-matmul partition reduction + Abs_reciprocal_sqrt +
      K=1 broadcast matmul; bias+normalize fused into one
      scalar_tensor_tensor per half (bf16 embeddings).
   2. A-matrix slices aT = (1-alpha) g0 + alpha g2 are built as each
      (g0, g2) pair chunk retires; alpha broadcast via gpsimd
      partition_broadcast. Per-seq dots (q, aa, score) via ones-matmul;
      self-dist s, numer, c0, c1, softplus head-tail term.
   3. Cross matmuls per pivot group (exact value-dependent bounds) are
      interleaved into the projection stream as each pool group's
      normalize retires: cross = A_g^T @ pool_g -> Max8 = this core's
      top-8 candidates per global row (stored as bf16).
   4. One packed AllGather of [scalars (200x5 f32); top8 (1600x8 bf16)]
      per rank; a tiny dummy AllGather at kernel start absorbs the
      first-collective entry latency off the critical path.
   5. Replicated final phase, batched over all 13 row tiles: dist =
      c1*cross + c0 (monotone per row), top-8 of 64, deno = numer +
      sum(exp(top6)) - exp(max(s, v6)) (arithmetic self-exclusion),
      means via ones-matmul; core 0's [1, 2] output is the answer.

The instruction stream depends on the bridge contents (group bounds),
so compiled NEFFs are cached keyed by a hash of the bridge tensor and
rebuilt if it changes.
"""

import hashlib
import sys

sys.path.insert(0, "/opt/trn_rl_repo")

import ml_dtypes
import numpy as np

import concourse.bacc as bacc
import concourse.bass as bass
import concourse.mybir as mybir
import concourse.tile as tile
from concourse.bass_utils import run_bass_kernel_spmd

F32 = mybir.dt.float32
F32R = mybir.dt.float32r
BF16 = mybir.dt.bfloat16
I32 = mybir.dt.int32
AF = mybir.ActivationFunctionType
OP = mybir.AluOpType
AX = mybir.AxisListType

BS, T, Q, HID, PROJ = 16, 16, 100, 256, 256
NSEQ = BS * Q              # 1600
NCORES = 8
SPC = NSEQ // NCORES       # 200 cur sequences per core
NG = T - 2                 # 14 pivot groups
HT2 = NSEQ                 # g2_all block start (g0_all at 0)
PV = 2 * NSEQ              # own pivot-frame block (200)
OG0 = PV + SPC             # own g0 copy (200)
OG2 = OG0 + SPC            # own g2 copy (200)
POOL0 = OG2 + SPC          # 3800: pool region start
C = POOL0 + 2 * SPC * NG   # 9400 columns per core
DELTA = 0.3
CHUNK = 512
ROWT = (NSEQ + 127) // 128  # 13 final-phase row tiles

RSPLIT = NSEQ             # AG2 split row (NSEQ = single collective)
CCA_LEN = SPC * 5 + 4 * RSPLIT       # scal (f32) + t8 rows (bf16 pairs)
CCB_LEN = 4 * (NSEQ - RSPLIT)


def _chunks():
    out = []
    s = 0
    while s < C:
        out.append((s, min(CHUNK, C - s)))
        s += CHUNK
    return out


def _build_fused(gbounds):
    """gbounds: list of (gs, ge) global sorted-row bounds per group g=1..14."""
    nc = bacc.Bacc("TRN2", target_bir_lowering=False, debug=False,
                   num_devices=NCORES)
    xt_in = nc.declare_dram_parameter("xt_in", [HID, C], BF16, isOutput=False)
    w_in = nc.declare_dram_parameter("w_in", [HID, PROJ], BF16, isOutput=False)
    b_in = nc.declare_dram_parameter("b_in", [HID, 1], F32, isOutput=False)
    brt_in = nc.declare_dram_parameter("brt_in", [3, SPC], I32, isOutput=False)
    brta_in = nc.declare_dram_parameter("brta_in", [3, NSEQ], I32,
                                        isOutput=False)
    ones_in = nc.declare_dram_parameter("ones_in", [2, 128], F32R,
                                        isOutput=False)
    out2 = nc.declare_dram_parameter("out2", [1, 2], F32, isOutput=True)

    # value-dependent M-tiles: (group g, global row start, row end)
    mtiles = []
    for g in range(1, NG + 1):
        gs, ge = gbounds[g - 1]
        ms = gs
        while ms < ge:
            me = min(ms + 128, ge)
            mtiles.append((g, ms, me))
            ms = me

    with tile.TileContext(nc) as tc:
        with (
            tc.tile_pool(name="singles", bufs=1) as singles,
            tc.tile_pool(name="work", bufs=4) as work,
            tc.tile_pool(name="fin", bufs=3) as finp,
            tc.tile_pool(name="dram", bufs=1, space="DRAM") as dram,
        ):
            engs = (nc.sync, nc.scalar, nc.gpsimd)

            # ---- activation table preheat (overlaps input DMA) ----
            dummy = singles.tile([1, 8], F32, tag="dummy")
            nc.vector.memset(dummy, 1.0)
            for fn in (AF.Exp, AF.Abs_reciprocal_sqrt):
                nc.scalar.activation(out=dummy, in_=dummy, func=fn)

            # ---- constants / small inputs ----
            w_sb = []
            for kt in range(2):
                t_w = singles.tile([128, PROJ], BF16, tag=f"w{kt}")
                engs[kt].dma_start(out=t_w, in_=w_in[kt * 128:(kt + 1) * 128, :])
                w_sb.append(t_w)
            ones128 = singles.tile([128, 1], F32R, tag="ones128")
            nc.gpsimd.dma_start(
                out=ones128,
                in_=ones_in[0:1, :].rearrange("x (b y) -> (x b) y", y=1))
            ones1 = singles.tile([1, 128], F32R, tag="ones1")
            nc.gpsimd.dma_start(out=ones1, in_=ones_in[0:1, :])
            delta1 = singles.tile([1, 1], F32, tag="delta1")
            nc.vector.memset(delta1, DELTA)
            b_sb = []
            for h in range(2):
                t_b = singles.tile([128, 1], F32, tag=f"b{h}")
                nc.gpsimd.dma_start(out=t_b, in_=b_in[h * 128:(h + 1) * 128, :])
                b_sb.append(t_b)

            # ---- xt input: graded slabs, both halves' heads first ----
            xt_sb = [singles.tile([128, C], BF16, tag=f"xt{h}", name=f"xt{h}")
                     for h in range(2)]
            SLABS = (512, 512, 1024, 1536, 1536, 1536, 1372, 1372)
            a = 0
            for si, w_s in enumerate(SLABS):
                bnd = min(a + w_s, C)
                for h in range(2):
                    eng = nc.gpsimd if si == 3 else engs[h]
                    eng.dma_start(
                        out=xt_sb[h][:, a:bnd],
                        in_=xt_in[h * 128:(h + 1) * 128, a:bnd])
                a = bnd

            emb = [singles.tile([128, C], BF16, tag=f"emb{h}", name=f"emb{h}")
                   for h in range(2)]
            aT_all = [singles.tile([128, NSEQ], BF16, tag=f"aTall{h}",
                                   name=f"aTall{h}") for h in range(2)]

            # dummy early AllGather: absorbs the (large, variable)
            # first-collective entry latency off the critical path; its
            # completion is never awaited by compute
            ccW_in = dram.tile([1, 32], F32, tag="ccWin")
            ccW_out = dram.tile([NCORES, 32], F32, tag="ccWout",
                                addr_space="Shared")
            warm = singles.tile([1, 32], F32, tag="warm")
            nc.vector.memset(warm, 0.0)
            nc.gpsimd.dma_start(out=ccW_in[:, :], in_=warm)
            nc.gpsimd.collective_compute(
                "AllGather", OP.bypass,
                replica_groups=[list(range(NCORES))],
                ins=[ccW_in[:, :].opt()],
                outs=[ccW_out[:, :].opt()])
            cc_inA = dram.tile([CCA_LEN, 1], F32, tag="ccinA")
            cc_outA = dram.tile([NCORES, CCA_LEN, 1], F32, tag="ccoutA",
                                addr_space="Shared")
            if RSPLIT < NSEQ:
                cc_inB = dram.tile([CCB_LEN, 1], F32, tag="ccinB")
                cc_outB = dram.tile([NCORES, CCB_LEN, 1], F32, tag="ccoutB",
                                    addr_space="Shared")

            # ---- per-seq alpha chain (needs only brt, runs before proj) ----
            bfi = singles.tile([1, 3 * SPC], I32, tag="bfi")
            for i in range(3):
                nc.gpsimd.dma_start(out=bfi[:, i * SPC:(i + 1) * SPC],
                                    in_=brt_in[i:i + 1, :])
            bf = singles.tile([1, 3 * SPC], F32, tag="bf")
            nc.vector.tensor_copy(out=bf, in_=bfi)
            bh, bp, bt = (bf[:, i * SPC:(i + 1) * SPC] for i in range(3))
            sc = singles.tile([1, 2600], F32, tag="sc")

            def R(i):
                return sc[:, i * SPC:(i + 1) * SPC]

            (alpha, rsg, s_sd, c0r, nmr, spr, tmp, sigr, qr, aar,
             scr) = (R(i) for i in range(11))

            def tt(o, i0, i1, op):
                nc.vector.tensor_tensor(out=o, in0=i0, in1=i1, op=op)

            tt(alpha, bp, bh, OP.subtract)
            tt(tmp, bt, bh, OP.subtract)
            nc.vector.reciprocal(out=tmp, in_=tmp)
            tt(alpha, alpha, tmp, OP.mult)
            tt(sigr, bt, bp, OP.subtract)
            tt(sigr, alpha, sigr, OP.mult)          # sigma
            tt(tmp, sigr, sigr, OP.mult)            # sigma^2
            nc.vector.reciprocal(out=rsg, in_=tmp)  # c1 = 1/sigma^2

            # global alpha over all 1600 sorted rows (identical on all
            # cores); bridge head/tail are 0/T-1 by construction, so
            # alpha = pivot/(T-1)
            bfa_i = work.tile([1, NSEQ], I32, tag="bfai", bufs=1)
            nc.gpsimd.dma_start(out=bfa_i, in_=brta_in[1:2, :])
            al_a = singles.tile([1, NSEQ], F32, tag="ala")
            nc.vector.tensor_copy(out=al_a, in_=bfa_i)
            nc.vector.tensor_scalar(out=al_a, in0=al_a,
                                    scalar1=1.0 / (T - 1), scalar2=None,
                                    op0=OP.mult)

            with tc.tile_pool(name="psA", bufs=1, space="PSUM") as psA:
                ab = singles.tile([128, SPC], F32, tag="ab", name="ab")
                nc.gpsimd.partition_broadcast(ab[:, :], alpha[:, :])
                ab_all = singles.tile([128, NSEQ], F32, tag="aball",
                                      name="ab_all")
                nc.gpsimd.partition_broadcast(ab_all[:, :], al_a[:, :])
                om_a = work.tile([1, NSEQ], F32, tag="oma", bufs=1)
                nc.vector.tensor_scalar(out=om_a, in0=al_a, scalar1=-1.0,
                                        scalar2=1.0, op0=OP.mult, op1=OP.add)
                omb_all = singles.tile([128, NSEQ], F32, tag="omball",
                                       name="omb_all")
                nc.gpsimd.partition_broadcast(omb_all[:, :], om_a[:, :])

                # software-pipelined projection: stage A (proj matmuls +
                # squares) for chunk c+1 is emitted before stage B
                # (colsum/rsqrt/bcast/normalize) of chunk c, so the PE
                # queue never blocks on the scalar engine.
                def stage_a(s, w):
                    ps_p = []
                    sq = []
                    for h in range(2):
                        pp = psA.tile([128, CHUNK], F32, tag=f"pp{h}", bufs=3 - h)
                        for kt in range(2):
                            nc.tensor.matmul(
                                out=pp[:, :w],
                                lhsT=w_sb[kt][:, h * 128:(h + 1) * 128],
                                rhs=xt_sb[kt][:, s:s + w],
                                start=(kt == 0), stop=(kt == 1))
                        sq_h = work.tile([128, CHUNK], F32R, tag=f"sq{h}", bufs=3)
                        nc.scalar.activation(out=sq_h[:, :w], in_=pp[:, :w],
                                             func=AF.Square, bias=b_sb[h])
                        ps_p.append(pp)
                        sq.append(sq_h)
                    return (s, w, ps_p, sq)

                def stage_b(st):
                    s, w, ps_p, sq = st
                    ss = psA.tile([1, CHUNK], F32, tag="ss", bufs=1)
                    for h in range(2):
                        nc.tensor.matmul(out=ss[:, :w], lhsT=ones128,
                                         rhs=sq[h][:, :w],
                                         start=(h == 0), stop=(h == 1))
                    rn = work.tile([1, CHUNK], F32, tag="rn", bufs=3)
                    nc.scalar.activation(out=rn[:, :w], in_=ss[:, :w],
                                         func=AF.Abs_reciprocal_sqrt)
                    rb_sb = work.tile([128, CHUNK], F32, tag="rbsb", bufs=3)
                    nc.gpsimd.partition_broadcast(rb_sb[:, :w], rn[:, :w])
                    with nc.allow_low_precision(reason="bf16 embeddings"):
                        for h in range(2):
                            nc.vector.scalar_tensor_tensor(
                                out=emb[h][:, s:s + w], in0=ps_p[h][:, :w],
                                scalar=b_sb[h], in1=rb_sb[:, :w],
                                op0=OP.add, op1=OP.mult)

                chunks = _chunks()
                # group g's pool columns end at POOL0 + 400g; map each
                # chunk to the groups whose pool it completes
                gdone = {}
                for g in range(1, NG + 1):
                    cidx = (POOL0 + 2 * SPC * g - 1) // CHUNK
                    gdone.setdefault(cidx, []).append(g)
                xei = 0

                def emit_cross(g):
                    nonlocal xei
                    for (gg, ms, me) in mtiles:
                        if gg != g:
                            continue
                        cw = me - ms
                        px = psA.tile([128, 2 * SPC], F32, tag="px", bufs=2,
                                      padded_shape=[128, CHUNK])
                        pool = POOL0 + 2 * SPC * (g - 1)
                        for h in range(2):
                            nc.tensor.matmul(
                                out=px[:cw, :],
                                lhsT=aT_all[h][:, ms:me],
                                rhs=emb[h][:, pool:pool + 2 * SPC],
                                start=(h == 0), stop=(h == 1))
                        t8t = work.tile([128, 8], BF16, tag="t8t")
                        nc.vector.max(out=t8t[:cw, :], in_=px[:cw, :])
                        engs[xei % 2].dma_start(
                            out=cc_inA[SPC * 5 + 4 * ms:SPC * 5 + 4 * me,
                                       0:1].rearrange(
                                           "(s e) x -> s (x e)",
                                           e=4).bitcast(BF16),
                            in_=t8t[:cw, :])
                        xei += 1

                pend = None
                aT = []
                for ci, (s, w) in enumerate(chunks):
                    cur = stage_a(s, w)
                    if pend is not None:
                        stage_b(pend)
                        bci = ci - 1
                        if bci >= 8:
                            for g in gdone.get(bci, []):
                                emit_cross(g)
                    pend = cur
                    if ci in (2, 4, 6, 7):
                        # the (g0, g2) pair covering aT columns
                        # [512j, 512j+PW) is normalized: build that slice of
                        # the global A matrix (small DVE ops, pipelined)
                        j = ci // 2 - 1 if ci < 7 else 3
                        a0 = 512 * j
                        pw = min(512, NSEQ - a0)
                        for h in range(2):
                            g0j = emb[h][:, 1024 * j:1024 * j + pw]
                            g2j = emb[h][:, 1024 * j + pw:1024 * j + 2 * pw]
                            da = work.tile([128, CHUNK], F32, tag=f"da{h}",
                                           bufs=2)
                            db = work.tile([128, CHUNK], F32, tag=f"db{h}",
                                           bufs=2)
                            tt(da[:, :pw], g0j,
                               omb_all[:, a0:a0 + pw], OP.mult)
                            tt(db[:, :pw], g2j,
                               ab_all[:, a0:a0 + pw], OP.mult)
                            with nc.allow_low_precision(reason="bf16 A"):
                                tt(aT_all[h][:, a0:a0 + pw], da[:, :pw],
                                   db[:, :pw], OP.add)
                    if ci == 8:
                        # own copies (cols 3200:3800) are normalized:
                        # per-seq dots and scalars
                        for h in range(2):
                            g0o = emb[h][:, OG0:OG0 + SPC]
                            g2o = emb[h][:, OG2:OG2 + SPC]
                            d = work.tile([128, SPC], F32, tag=f"ad{h}", bufs=1)
                            tt(d, g2o, g0o, OP.subtract)
                            a_h = singles.tile([128, SPC], F32R, tag=f"aT{h}",
                                               name=f"aTh{h}")
                            nc.vector.tensor_tensor(out=a_h, in0=d, in1=ab,
                                                    op=OP.mult)
                            tt(a_h, a_h, g0o, OP.add)
                            aT.append(a_h)
                        # dots: q = a.g1, aa = a.a, score = g0.g2
                        for di, (f0, f1) in enumerate((
                            (lambda h: aT[h],
                             lambda h: emb[h][:, PV:PV + SPC]),
                            (lambda h: aT[h], lambda h: aT[h]),
                            (lambda h: emb[h][:, OG0:OG0 + SPC],
                             lambda h: emb[h][:, OG2:OG2 + SPC]),
                        )):
                            dp_t = psA.tile([1, CHUNK], F32, tag="ss",
                                            bufs=1, name="dp_t")
                            dp = dp_t[:, :SPC]
                            for h in range(2):
                                pr = work.tile([128, SPC], F32R, tag=f"pr{h}", bufs=2)
                                tt(pr, f0(h), f1(h), OP.mult)
                                nc.tensor.matmul(out=dp, lhsT=ones128,
                                                 rhs=pr,
                                                 start=(h == 0), stop=(h == 1))
                            nc.vector.tensor_copy(out=R(8 + di), in_=dp)

                        # s = (2q - 1 - aa)/(2 sigma^2)
                        nc.vector.tensor_scalar(out=tmp, in0=qr, scalar1=2.0,
                                                scalar2=-1.0, op0=OP.mult,
                                                op1=OP.add)
                        tt(tmp, tmp, aar, OP.subtract)
                        tt(tmp, tmp, rsg, OP.mult)
                        nc.vector.tensor_scalar(out=s_sd, in0=tmp, scalar1=0.5,
                                                scalar2=None, op0=OP.mult)
                        # c0 = -(1 + aa)/(2 sigma^2)
                        nc.vector.tensor_scalar(out=tmp, in0=aar, scalar1=1.0,
                                                scalar2=None, op0=OP.add)
                        tt(tmp, tmp, rsg, OP.mult)
                        nc.vector.tensor_scalar(out=c0r, in0=tmp, scalar1=-0.5,
                                                scalar2=None, op0=OP.mult)
                if pend is not None:
                    stage_b(pend)
                    for g in gdone.get(len(chunks) - 1, []):
                        emit_cross(g)

                # exp/ln work deferred here so the scalar engine never swaps
                # activation tables mid-projection
                nc.scalar.activation(out=nmr, in_=s_sd, func=AF.Exp)
                nc.scalar.activation(out=spr, in_=scr, func=AF.Exp,
                                     scale=-1.0, bias=delta1)
                nc.scalar.activation(out=spr, in_=spr, func=AF.Ln,
                                     bias=1.0)
                scv = cc_inA[0:SPC * 5, 0:1].rearrange(
                    "(s e) x -> e (s x)", e=5)
                for qi, row in enumerate((c0r, rsg, s_sd, nmr, spr)):
                    nc.scalar.dma_start(out=scv[qi:qi + 1, :], in_=row)

            with tc.tile_pool(name="psB", bufs=1, space="PSUM") as psB:
                # ---- AllGather 2 (cross already ran inline above) ----
                nc.gpsimd.collective_compute(
                    "AllGather", OP.bypass,
                    replica_groups=[list(range(NCORES))],
                    ins=[cc_inA[:, :].opt()],
                    outs=[cc_outA[:, :, :].opt()])

                # ---- final phase (replicated, batched over row tiles) ----
                cand_all = singles.tile([128, ROWT, 64], BF16, tag="candall")
                sct_all = singles.tile([128, ROWT, 5], F32, tag="sctall")
                nc.vector.memset(cand_all, 0.0)
                nc.vector.memset(sct_all, 0.0)

                def cc_view(t):
                    r0 = 128 * t
                    psz = min(128, NSEQ - r0)
                    return cc_outA[:, SPC * 5 + 4 * r0:
                                   SPC * 5 + 4 * (r0 + psz), 0:1].rearrange(
                                       "k (s e) x -> s k (e x)",
                                       e=4).bitcast(BF16)
                ei = 0
                for t in range(ROWT):
                    r0 = 128 * t
                    psz = min(128, NSEQ - r0)
                    engs[ei % 3].dma_start(
                        out=cand_all[:psz, t, :].rearrange(
                            "s (k e) -> s k e", k=8),
                        in_=cc_view(t))
                    ei += 1
                    k0 = r0 // SPC
                    k1 = (r0 + psz - 1) // SPC
                    for k in range(k0, k1 + 1):
                        a = max(r0, SPC * k)
                        bnd = min(r0 + psz, SPC * (k + 1))
                        engs[ei % 3].dma_start(
                            out=sct_all[a - r0:bnd - r0, t, :],
                            in_=cc_outA[k, (a - SPC * k) * 5:
                                        (bnd - SPC * k) * 5,
                                        0:1].rearrange("(s e) x -> s (x e)", e=5))
                        ei += 1
                c0b = sct_all[:, :, 0:1].to_broadcast([128, ROWT, 64])
                c1b = sct_all[:, :, 1:2].to_broadcast([128, ROWT, 64])
                d_all = singles.tile([128, ROWT, 64], F32, tag="dall")
                nc.vector.tensor_tensor(out=d_all, in0=cand_all, in1=c1b,
                                        op=OP.mult)
                nc.vector.tensor_tensor(out=d_all, in0=d_all, in1=c0b,
                                        op=OP.add)
                t8a = singles.tile([128, ROWT, 8], F32, tag="t8a")
                for t in range(ROWT):
                    nc.vector.max(out=t8a[:, t, :], in_=d_all[:, t, :])
                e6 = singles.tile([128, ROWT, 6], F32, tag="e6")
                nc.scalar.activation(out=e6, in_=t8a[:, :, 0:6], func=AF.Exp)
                se = singles.tile([128, ROWT], F32, tag="se")
                nc.vector.reduce_sum(out=se[:, :].unsqueeze(-1), in_=e6,
                                     axis=AX.X)
                mx = singles.tile([128, ROWT], F32, tag="mx")
                nc.vector.tensor_tensor(out=mx[:, :].unsqueeze(-1),
                                        in0=t8a[:, :, 5:6],
                                        in1=sct_all[:, :, 2:3], op=OP.max)
                em = singles.tile([128, ROWT], F32, tag="em")
                nc.scalar.activation(out=em, in_=mx, func=AF.Exp)
                nmv = sct_all[:, :, 3]
                nc.vector.tensor_tensor(out=se, in0=se, in1=em, op=OP.subtract)
                nc.vector.tensor_tensor(out=se[:, :].unsqueeze(-1),
                                        in0=se[:, :].unsqueeze(-1),
                                        in1=sct_all[:, :, 3:4], op=OP.add)
                nc.vector.reciprocal(out=se, in_=se)
                lsum = singles.tile([128, ROWT], F32, tag="lsum")
                nc.vector.tensor_tensor(out=lsum[:, :].unsqueeze(-1),
                                        in0=sct_all[:, :, 3:4],
                                        in1=se[:, :].unsqueeze(-1), op=OP.mult)

                red = singles.tile([128, 2], F32R, tag="red")
                with nc.allow_low_precision(reason="f32r is f32 bits"):
                    nc.vector.reduce_sum(out=red[:, 0:1], in_=lsum, axis=AX.X)
                    nc.vector.reduce_sum(out=red[:, 1:2],
                                     in_=sct_all[:, :, 4:5].rearrange(
                                         "p t x -> p (t x)"), axis=AX.X)
                fin_ps = psB.tile([1, 2], F32, tag="finps", bufs=1,
                                  padded_shape=[1, CHUNK])
                nc.tensor.matmul(out=fin_ps, lhsT=ones128,
                                 rhs=red,
                                 start=True, stop=True)
                fin = singles.tile([1, 2], F32, tag="fin")
                nc.vector.tensor_scalar(out=fin, in0=fin_ps,
                                        scalar1=1.0 / NSEQ,
                                        scalar2=None, op0=OP.mult)
                nc.sync.dma_start(out=out2[:, :], in_=fin)
    nc.compile()
    return nc


_NC_CACHE = {}
LAST_RUNS = []


def _hw_runner(nc, in_maps):
    import os
    res = run_bass_kernel_spmd(
        nc, in_maps, list(range(NCORES)),
        trace=bool(os.environ.get("KERNEL_TRACE")))
    LAST_RUNS.append(res)
    return res.results


def kernel(frame_embeds, other_frame_embeds, W, b, bridge, _runner=None):
    frame_embeds = np.asarray(frame_embeds, dtype=np.float32)
    other_frame_embeds = np.asarray(other_frame_embeds, dtype=np.float32)
    W = np.ascontiguousarray(np.asarray(W, dtype=np.float32))
    b = np.asarray(b, dtype=np.float32)
    bridge = np.asarray(bridge, dtype=np.int32)

    runner = _runner if _runner is not None else _hw_runner

    # ---- host-side sharding / layout (pure indexing) ----
    fe_seq = frame_embeds.transpose(0, 2, 1, 3).reshape(NSEQ, T, HID)
    ofe_seq = other_frame_embeds.transpose(0, 2, 1, 3).reshape(NSEQ, T, HID)
    perm = np.argsort(bridge[:, 1], kind="stable")
    bridge_s = bridge[perm]

    piv = bridge_s[:, 1].astype(np.int64)
    counts = np.bincount(piv, minlength=T)[1:T - 1]
    gb = np.zeros(NG + 1, dtype=np.int64)
    gb[1:] = np.cumsum(counts)
    gbounds = [(int(gb[g - 1]), int(gb[g])) for g in range(1, NG + 1)]

    key = ("fused", hashlib.sha1(bridge.tobytes()).hexdigest())
    if key not in _NC_CACHE:
        _NC_CACHE[key] = _build_fused(gbounds)
    nc = _NC_CACHE[key]

    b_col = np.ascontiguousarray(b.reshape(HID, 1))
    ones_host = np.ones((2, 128), np.float32)
    W_bf = W.astype(ml_dtypes.bfloat16)
    fe_sorted = fe_seq[perm]                         # (1600, 16, 256)
    g0_all = fe_sorted[:, 0, :].T                    # (256, 1600)
    g2_all = fe_sorted[:, T - 1, :].T
    brtA = np.ascontiguousarray(bridge_s.T)          # (3, 1600)
    in_maps = []
    for k in range(NCORES):
        sl = slice(k * SPC, (k + 1) * SPC)
        fe_k = fe_sorted[sl]                         # (200, 16, 256)
        cur_t = fe_k.transpose(2, 1, 0)              # (256, 16, 200)
        oth_t = ofe_seq[sl].transpose(2, 1, 0)
        X = np.empty((HID, C), np.float32)
        for j in range(4):
            a0 = 512 * j
            pw = min(512, NSEQ - a0)
            X[:, 1024 * j:1024 * j + pw] = g0_all[:, a0:a0 + pw]
            X[:, 1024 * j + pw:1024 * j + 2 * pw] = g2_all[:, a0:a0 + pw]
        X[:, PV:PV + SPC] = fe_k[np.arange(SPC), bridge_s[sl, 1]].T
        X[:, OG0:OG0 + SPC] = cur_t[:, 0]
        X[:, OG2:OG2 + SPC] = cur_t[:, T - 1]
        for g in range(1, NG + 1):
            base = POOL0 + 2 * SPC * (g - 1)
            X[:, base:base + SPC] = cur_t[:, g]
            X[:, base + SPC:base + 2 * SPC] = oth_t[:, g]
        brT = np.ascontiguousarray(bridge_s[sl].T)
        in_maps.append({"xt_in": X.astype(ml_dtypes.bfloat16), "w_in": W_bf,
                        "b_in": b_col, "brt_in": brT, "brta_in": brtA,
                        "ones_in": ones_host})

    res = runner(nc, in_maps)
    out = res[0]["out2"]
    return (np.asarray(np.float32(out[0, 0])), np.asarray(np.float32(out[0, 1])))
